# revision 5
# baseline (speedup 1.0000x reference)
"""Chamfer distance kernel for Trainium2 (8 NeuronCores, data-parallel over batch).

reference:
    dist[b,i,j] = |x_bi|^2 + |y_bj|^2 - 2<x_bi, y_bj>
    out = mean_b,j( min_i dist ) + mean_b,i( min_j dist )

Device algorithm (per core = one batch):
  PE: raw distance blocks via a single K=13 fp16 matmul over two-way-split
  features (fp16 streams ~4x faster than fp32 and the split keeps ~fp32
  accuracy; the dropped x2*y2 term is O(|x||y| 2^-24)):
      x = x1 + x2 (fp16 hi/lo), s = |x|^2 = s1 + s2, same for y/t.
      lhsT rows (stationary): [x1(3), x1(3), x2(3), s1, s2, 1, 1]
      rhs  rows (moving):     [-2y1(3), -2y2(3), -2y1(3), 1, 1, t1, t2]
      => lhsT.T @ rhs = s + t - 2(x1y1 + x1y2 + x2y1) ~= dist  (fp32 PSUM)
  Consecutive [128,1024] blocks alternate PE row groups 0/32 (features are
  staged at SBUF partition offsets 0 and 32) so weight loads and matmul
  streaming overlap across groups -- measured ~120ns per 2-matmul block.

  Reduction (min over the free axis, both orientations so each of the two
  mins is free-axis only):
    - ScalarE cast-copies each PSUM block to fp16 SBUF (fp16 is safe: rounding
      is monotone so min commutes with it, and the result only needs 2e-2).
    - VectorE folds the 8 blocks of a row-chunk with a fp16 tensor_tensor min
      tree (fp16 SBUF pairs hit the DVE 2x_1P mode) into one [128,1024] tile
      per 128-point chunk, stashed into a [128, 8, 1024] buffer.
    - Every 8 chunks one strided tensor_reduce produces [128, 8] mins which
      ScalarE copies into the fp32 output strip.
  Host sums the [128, 128] strip.
"""

import numpy as np

import concourse.bass as bass
import concourse.tile as tile
import concourse.mybir as mybir
from concourse.bass_utils import run_bass_kernel_spmd
from concourse.vector_clock import ScopedClock

B, N, M, D = 8, 8192, 8192, 3
N_CORES = 8
FD = 512    # free-dim elements per matmul = one PSUM bank of fp32
BW = 1024   # block width (columns) = one [128,1024] PSUM tile = 2 matmuls
K = 13      # contraction dim of the split-feature matmul
GRP = 8     # row-chunks per strided-reduce batch


# --- workaround: this walrus build accepts only 1 sync-wait per instruction;
# split excess waits onto single-wait NoOps emitted on the same engine just
# before the offending instruction (per-engine program order preserves the
# semantics: all waits complete before the instruction issues).
_orig_add_instruction = tile.TileContext._add_instruction


def _add_instruction_split(self, inst):
    si = inst.sync_info
    if si is not None and len(si.on_wait) > 1:
        waits = list(si.on_wait)
        inst.sync_info = mybir.SyncInfo(on_wait=[waits[-1]], on_update=list(si.on_update))
        eng = self.nc.engines[inst.engine]
        for w in waits[:-1]:
            nop = eng.nop(nofuse=True)
            nop.ins.sync_info = mybir.SyncInfo(on_wait=[w], on_update=[])
    _orig_add_instruction(self, inst)


tile.TileContext._add_instruction = _add_instruction_split


def _drain_and_barrier_split(self, tick_clock, wait_clock):
    nc = self.nc
    probe = nc.sync.nop(nofuse=True)
    wait_clock.add_sem_waits(probe.ins, ScopedClock({None: tick_clock.global_clock}))
    si = probe.ins.sync_info
    waits = list(si.on_wait) if si is not None else []
    upds = list(si.on_update) if si is not None else []
    probe.ins.sync_info = mybir.SyncInfo(on_wait=waits[:1], on_update=upds)
    for w in waits[1:]:
        nop = nc.sync.nop(nofuse=True)
        nop.ins.sync_info = mybir.SyncInfo(on_wait=[w], on_update=[])
    nc.sync.drain()
    nc.all_engine_barrier()
    assert self.sems is not None
    popped = nc._tile_sem_poison_stack.pop()
    assert popped is self._sem_poison
    nc.clear_and_free_semaphores(list(self.sems.allocated().values()))
    nc.all_engine_barrier()


tile.TileContext._drain_and_barrier = _drain_and_barrier_split


def build_nc(n=N, m=M, repeat=1):
    """Bass program for one core: one batch of chamfer(n x-points, m y-points).

    Inputs (per orientation o in {a: x-partitions, b: y-partitions}):
      l_{o}: [K, n] fp16 stationary split features
      r_{o}: [K, m] fp16 moving split features
    Output: strip [128, 2 * n_xb]; strip[p, o*n_xb + xb] = min over all
    opposite-side points for point index xb*128 + p.
    """
    assert n == m, "loop/strip layout assumes equal point counts"
    assert n % 128 == 0 and m % (2 * BW) == 0 and (n // 128) % GRP == 0
    f16 = mybir.dt.float16
    f32 = mybir.dt.float32
    mn = mybir.AluOpType.min
    n_xb = n // 128        # 128-point chunks on the partition side
    n_tl = m // (2 * BW)   # [128,2048] PSUM tiles along the free side (4)
    assert n_tl == 4, "min-tree below is written for 4 tiles per chunk"

    nc = bass.Bass()
    params = {}
    for o in ("a", "b"):
        params[f"l_{o}"] = nc.declare_dram_parameter(f"l_{o}", [K, n], f16, isOutput=False)
        params[f"r_{o}"] = nc.declare_dram_parameter(f"r_{o}", [K, m], f16, isOutput=False)
    out = nc.declare_dram_parameter("strip", [128, 2 * n_xb], f32, isOutput=True)

    with tile.TileContext(nc) as tc:
        with (
            tc.tile_pool(name="inputs", bufs=1) as in_pool,
            tc.tile_pool(name="ps", bufs=2, space="PSUM") as ps_pool,
            tc.tile_pool(name="cp", bufs=10) as cp_pool,
            tc.tile_pool(name="l0", bufs=6) as l0_pool,
            tc.tile_pool(name="l1", bufs=4) as l1_pool,
            tc.tile_pool(name="wide", bufs=2) as wide_pool,
            tc.tile_pool(name="r8", bufs=2) as r8_pool,
            tc.tile_pool(name="strip", bufs=1) as strip_pool,
        ):
            # features staged twice: partitions 0..12 (PE row group 0) and
            # 32..44 (row group 1) -- walrus requires weights and moving
            # operand to start at the same partition.
            sb = {}
            for o in ("a", "b"):
                lt = in_pool.tile([45, n], f16, tag=f"l_{o}")
                rt = in_pool.tile([45, m], f16, tag=f"r_{o}")
                for g in (0, 32):
                    nc.sync.dma_start(lt[g:g + K, :], params[f"l_{o}"][:])
                    nc.sync.dma_start(rt[g:g + K, :], params[f"r_{o}"][:])
                sb[o] = (lt, rt)

            strip_t = strip_pool.tile([128, 2 * n_xb], f32)

            def produce(o, xb):
                """PE: 4 [128,2048] distance tiles (8 MM pairs, row groups
                alternating); ACT: one fp32->fp16 cast-copy per tile."""
                lt, rt = sb[o]
                cps = []
                for tl in range(n_tl):
                    p_ps = ps_pool.tile([128, 2 * BW], f32, name="pp", tag="p")
                    for h in range(2):
                        g = 32 * ((2 * tl + h) % 2)
                        lhs = lt[g:g + K, 128 * xb:128 * (xb + 1)]
                        c = 2 * BW * tl + BW * h
                        nc.tensor.matmul(p_ps[:, BW * h:BW * h + FD], lhs,
                                         rt[g:g + K, c:c + FD],
                                         start=True, stop=True, tile_position=(g, 0))
                        nc.tensor.matmul(p_ps[:, BW * h + FD:BW * (h + 1)], lhs,
                                         rt[g:g + K, c + FD:c + BW],
                                         start=True, stop=True, tile_position=(g, 0))
                    cp_t = cp_pool.tile([128, 2 * BW], f16, name="cp", tag="cp")
                    nc.scalar.copy(cp_t[:], p_ps[:])
                    cps.append(cp_t)
                return cps

            def reduce_chunk(oi, xb, cps, wide_t):
                """DVE: fold 4 [128,2048] fp16 tiles into wide_t[:, xb%GRP, :]
                via a tensor_tensor min tree (fp16 SBUF pairs -> 2x mode)."""
                l0s = []
                for tl in range(n_tl):
                    l0_t = l0_pool.tile([128, BW], f16, name="l0", tag="l0")
                    nc.vector.tensor_tensor(l0_t[:], cps[tl][:, 0:BW],
                                            cps[tl][:, BW:2 * BW], op=mn)
                    l0s.append(l0_t)
                l1a = l1_pool.tile([128, BW], f16, name="l1a", tag="l1")
                nc.vector.tensor_tensor(l1a[:], l0s[0][:], l0s[1][:], op=mn)
                l1b = l1_pool.tile([128, BW], f16, name="l1b", tag="l1")
                nc.vector.tensor_tensor(l1b[:], l0s[2][:], l0s[3][:], op=mn)
                nc.vector.tensor_tensor(wide_t[:, xb % GRP, :], l1a[:], l1b[:], op=mn)
                if xb % GRP == GRP - 1:
                    r8_t = r8_pool.tile([128, GRP], f16, name="r8", tag="r8")
                    nc.vector.tensor_reduce(r8_t[:, :, None], wide_t[:],
                                            axis=mybir.AxisListType.X, op=mn)
                    col = oi * n_xb + (xb - GRP + 1)
                    nc.scalar.copy(strip_t[:, col:col + GRP], r8_t[:])

            for _rep in range(repeat):
                # software pipeline: emit produce(xb) before reduce(xb-1) so
                # ScalarE copies of chunk xb overlap the VectorE tree of xb-1.
                units = [(oi, o, xb) for oi, o in enumerate(("a", "b"))
                         for xb in range(n_xb)]
                wide = {}
                pend = None  # (oi, xb, cps, wide_t)
                for oi, o, xb in units:
                    if xb % GRP == 0:
                        wide[oi] = wide_pool.tile([128, GRP, BW], f16,
                                                  name="wd", tag="wd")
                    cps = produce(o, xb)
                    if pend is not None:
                        reduce_chunk(pend[0], pend[1], pend[2], pend[3])
                    pend = (oi, xb, cps, wide[oi])
                reduce_chunk(pend[0], pend[1], pend[2], pend[3])

            nc.sync.dma_start(out[:], strip_t[:])
    return nc


def _split16(v):
    """fp64 array -> (hi, lo) fp16 arrays with hi + lo ~= v."""
    hi = v.astype(np.float16)
    lo = (v - hi.astype(np.float64)).astype(np.float16)
    return hi, lo


def _features(pts, stationary):
    """pts [n,3] float64 -> [13, n] fp16 feature rows.

    stationary: [x1(3), x1(3), x2(3), s1, s2, 1, 1]
    moving:     [-2y1(3), -2y2(3), -2y1(3), 1, 1, t1, t2]
    """
    n = pts.shape[0]
    p1, p2 = _split16(pts)            # [n,3] each
    sq = np.sum(pts * pts, axis=-1)   # [n]
    s1, s2 = _split16(sq)
    one = np.ones(n, np.float16)
    if stationary:
        rows = [p1[:, 0], p1[:, 1], p1[:, 2],
                p1[:, 0], p1[:, 1], p1[:, 2],
                p2[:, 0], p2[:, 1], p2[:, 2],
                s1, s2, one, one]
    else:
        m2p1 = -2.0 * p1  # exact in fp16 (power-of-two scale)
        m2p2 = -2.0 * p2
        rows = [m2p1[:, 0], m2p1[:, 1], m2p1[:, 2],
                m2p2[:, 0], m2p2[:, 1], m2p2[:, 2],
                m2p1[:, 0], m2p1[:, 1], m2p1[:, 2],
                one, one, s1, s2]
    return np.ascontiguousarray(np.stack(rows), np.float16)


def make_in_map(xb, yb):
    """Per-core input map from one batch xb [n,3], yb [m,3]."""
    xb = np.asarray(xb, np.float64)
    yb = np.asarray(yb, np.float64)
    return {
        "l_a": _features(xb, True),
        "r_a": _features(yb, False),
        "l_b": _features(yb, True),
        "r_b": _features(xb, False),
    }


_NC_CACHE = {}


def _get_nc(n, m):
    key = (n, m)
    if key not in _NC_CACHE:
        _NC_CACHE[key] = build_nc(n, m)
    return _NC_CACHE[key]


def run_device(x, y, trace=False):
    """x [B,n,3], y [B,m,3] -> BassKernelResults with per-core strips."""
    n, m = x.shape[1], y.shape[1]
    assert x.shape[0] == N_CORES and y.shape[0] == N_CORES
    nc = _get_nc(n, m)
    in_maps = [make_in_map(x[b], y[b]) for b in range(x.shape[0])]
    return run_bass_kernel_spmd(nc, in_maps, list(range(N_CORES)), trace=trace)


def kernel(x, y):
    x = np.asarray(x)
    y = np.asarray(y)
    n, m = x.shape[1], y.shape[1]
    n_xb = n // 128
    res = run_device(x, y)
    s2_tot = 0.0  # sum over per-x mins  (reference dist2, min over j)
    s1_tot = 0.0  # sum over per-y mins  (reference dist1, min over i)
    for b in range(x.shape[0]):
        strip = res.results[b]["strip"].astype(np.float64)
        s2_tot += strip[:, :n_xb].sum()
        s1_tot += strip[:, n_xb:].sum()
    out = s1_tot / (x.shape[0] * m) + s2_tot / (x.shape[0] * n)
    return np.float32(out)


# revision 12
# speedup vs baseline: 94.5418x; 94.5418x over previous
"""Chamfer distance kernel for Trainium2 (8 NeuronCores, data-parallel over batch).

reference:
    dist[b,i,j] = |x_bi|^2 + |y_bj|^2 - 2<x_bi, y_bj>
    out = mean_b,j( min_i dist ) + mean_b,i( min_j dist )

Device algorithm (per core = one batch):
  PE: raw distance blocks via a single K=13 fp16 matmul over two-way-split
  features (fp16 streams ~4x faster than fp32 and the split keeps ~fp32
  accuracy; the dropped x2*y2 term is O(|x||y| 2^-24)):
      x = x1 + x2 (fp16 hi/lo), s = |x|^2 = s1 + s2, same for y/t.
      lhsT rows (stationary): [x1(3), x1(3), x2(3), s1, s2, 1, 1]
      rhs  rows (moving):     [-2y1(3), -2y2(3), -2y1(3), 1, 1, t1, t2]
      => lhsT.T @ rhs = s + t - 2(x1y1 + x1y2 + x2y1) ~= dist  (fp32 PSUM)
  Consecutive [128,1024] blocks alternate PE row groups 0/32 (features are
  staged at SBUF partition offsets 0 and 32) so weight loads and matmul
  streaming overlap across groups -- measured ~120ns per 2-matmul block.

  Reduction (min over the free axis, both orientations so each of the two
  mins is free-axis only):
    - ScalarE cast-copies each PSUM block to fp16 SBUF (fp16 is safe: rounding
      is monotone so min commutes with it, and the result only needs 2e-2).
    - VectorE folds the 8 blocks of a row-chunk with a fp16 tensor_tensor min
      tree (fp16 SBUF pairs hit the DVE 2x_1P mode) into one [128,1024] tile
      per 128-point chunk, stashed into a [128, 8, 1024] buffer.
    - Every 8 chunks one strided tensor_reduce produces [128, 8] mins which
      ScalarE copies into the fp32 output strip.
  Host sums the [128, 128] strip.
"""

import numpy as np

import concourse.bass as bass
import concourse.tile as tile
import concourse.mybir as mybir
from concourse.bass_utils import run_bass_kernel_spmd
from concourse.vector_clock import ScopedClock

B, N, M, D = 8, 8192, 8192, 3
N_CORES = 8
FD = 512    # free-dim elements per matmul = one PSUM bank of fp32
BW = 1024   # block width (columns) = one [128,1024] PSUM tile = 2 matmuls
K = 13      # contraction dim of the split-feature matmul
GRP = 8     # row-chunks per strided-reduce batch


# --- workaround: this walrus build accepts only 1 sync-wait per instruction;
# split excess waits onto single-wait NoOps emitted on the same engine just
# before the offending instruction (per-engine program order preserves the
# semantics: all waits complete before the instruction issues).
_orig_add_instruction = tile.TileContext._add_instruction


def _add_instruction_split(self, inst):
    si = inst.sync_info
    if si is not None and len(si.on_wait) > 1:
        waits = list(si.on_wait)
        inst.sync_info = mybir.SyncInfo(on_wait=[waits[-1]], on_update=list(si.on_update))
        eng = self.nc.engines[inst.engine]
        for w in waits[:-1]:
            nop = eng.nop(nofuse=True)
            nop.ins.sync_info = mybir.SyncInfo(on_wait=[w], on_update=[])
    _orig_add_instruction(self, inst)


tile.TileContext._add_instruction = _add_instruction_split


def _drain_and_barrier_split(self, tick_clock, wait_clock):
    nc = self.nc
    probe = nc.sync.nop(nofuse=True)
    wait_clock.add_sem_waits(probe.ins, ScopedClock({None: tick_clock.global_clock}))
    si = probe.ins.sync_info
    waits = list(si.on_wait) if si is not None else []
    upds = list(si.on_update) if si is not None else []
    probe.ins.sync_info = mybir.SyncInfo(on_wait=waits[:1], on_update=upds)
    for w in waits[1:]:
        nop = nc.sync.nop(nofuse=True)
        nop.ins.sync_info = mybir.SyncInfo(on_wait=[w], on_update=[])
    nc.sync.drain()
    nc.all_engine_barrier()
    assert self.sems is not None
    popped = nc._tile_sem_poison_stack.pop()
    assert popped is self._sem_poison
    nc.clear_and_free_semaphores(list(self.sems.allocated().values()))
    nc.all_engine_barrier()


tile.TileContext._drain_and_barrier = _drain_and_barrier_split


def build_nc(n=N, m=M, repeat=1):
    """Bass program for one core: one batch of chamfer(n x-points, m y-points).

    Inputs (per orientation o in {a: x-partitions, b: y-partitions}):
      l_{o}: [K, n] fp16 stationary split features
      r_{o}: [K, m] fp16 moving split features
    Output: strip [128, 2 * n_xb]; strip[p, o*n_xb + xb] = min over all
    opposite-side points for point index xb*128 + p.
    """
    assert n == m, "loop/strip layout assumes equal point counts"
    assert n % 128 == 0 and m % (2 * BW) == 0 and (n // 128) % GRP == 0
    f16 = mybir.dt.float16
    f32 = mybir.dt.float32
    mn = mybir.AluOpType.min
    n_xb = n // 128        # 128-point chunks on the partition side
    n_tl = m // (2 * BW)   # [128,2048] PSUM tiles along the free side (4)
    assert n_tl == 4, "min-tree below is written for 4 tiles per chunk"

    nc = bass.Bass()
    params = {}
    for o in ("a", "b"):
        params[f"l_{o}"] = nc.declare_dram_parameter(f"l_{o}", [K, n], f16, isOutput=False)
        params[f"r_{o}"] = nc.declare_dram_parameter(f"r_{o}", [K, m], f16, isOutput=False)
    out = nc.declare_dram_parameter("strip", [128, 2 * n_xb], f32, isOutput=True)

    with tile.TileContext(nc) as tc:
        with (
            tc.tile_pool(name="inputs", bufs=1) as in_pool,
            tc.tile_pool(name="ps", bufs=2, space="PSUM") as ps_pool,
            tc.tile_pool(name="cp", bufs=6) as cp_pool,
            tc.tile_pool(name="l0", bufs=6) as l0_pool,
            tc.tile_pool(name="l1", bufs=4) as l1_pool,
            tc.tile_pool(name="wide", bufs=2) as wide_pool,
            tc.tile_pool(name="r8", bufs=2) as r8_pool,
            tc.tile_pool(name="strip", bufs=1) as strip_pool,
        ):
            # features staged twice: partitions 0..12 (PE row group 0) and
            # 32..44 (row group 1) -- walrus requires weights and moving
            # operand to start at the same partition.
            sb = {}
            for o in ("a", "b"):
                lt = in_pool.tile([45, n], f16, tag=f"l_{o}")
                rt = in_pool.tile([45, m], f16, tag=f"r_{o}")
                for g in (0, 32):
                    nc.sync.dma_start(lt[g:g + K, :], params[f"l_{o}"][:])
                    nc.sync.dma_start(rt[g:g + K, :], params[f"r_{o}"][:])
                sb[o] = (lt, rt)

            strip_t = strip_pool.tile([128, 2 * n_xb], f32)

            def produce(o, xb):
                """PE: 4 [128,2048] distance tiles (8 MM pairs, row groups
                alternating); ACT: one fp32->fp16 cast-copy per tile, two
                tiles packed into one [128,4096] fp16 buffer."""
                lt, rt = sb[o]
                cps = []
                for half in range(2):
                    cp_t = cp_pool.tile([128, 4 * BW], f16, name="cp", tag="cp")
                    for tl2 in range(2):
                        tl = 2 * half + tl2
                        p_ps = ps_pool.tile([128, 2 * BW], f32, name="pp", tag="p")
                        for h in range(2):
                            g = 32 * ((2 * tl + h) % 2)
                            lhs = lt[g:g + K, 128 * xb:128 * (xb + 1)]
                            c = 2 * BW * tl + BW * h
                            nc.tensor.matmul(p_ps[:, BW * h:BW * h + FD], lhs,
                                             rt[g:g + K, c:c + FD],
                                             start=True, stop=True, tile_position=(g, 0))
                            nc.tensor.matmul(p_ps[:, BW * h + FD:BW * (h + 1)], lhs,
                                             rt[g:g + K, c + FD:c + BW],
                                             start=True, stop=True, tile_position=(g, 0))
                        nc.scalar.copy(cp_t[:, 2 * BW * tl2:2 * BW * (tl2 + 1)],
                                       p_ps[:])
                    cps.append(cp_t)
                return cps

            def reduce_chunk(oi, xb, cps, wide_t):
                """DVE: fold two [128,4096] fp16 buffers into wide_t[:, xb%GRP, :]
                with 3 half-folds + 1 quarter-fold (fp16 SBUF pairs -> 2x mode)."""
                oa = l0_pool.tile([128, 2 * BW], f16, name="oa", tag="l0")
                nc.vector.tensor_tensor(oa[:], cps[0][:, 0:2 * BW],
                                        cps[0][:, 2 * BW:4 * BW], op=mn)
                ob = l0_pool.tile([128, 2 * BW], f16, name="ob", tag="l0")
                nc.vector.tensor_tensor(ob[:], cps[1][:, 0:2 * BW],
                                        cps[1][:, 2 * BW:4 * BW], op=mn)
                w2 = l1_pool.tile([128, 2 * BW], f16, name="w2", tag="l1")
                nc.vector.tensor_tensor(w2[:], oa[:], ob[:], op=mn)
                nc.vector.tensor_tensor(wide_t[:, xb % GRP, :], w2[:, 0:BW],
                                        w2[:, BW:2 * BW], op=mn)
                if xb % GRP == GRP - 1:
                    r8_t = r8_pool.tile([128, GRP], f16, name="r8", tag="r8")
                    nc.vector.tensor_reduce(r8_t[:, :, None], wide_t[:],
                                            axis=mybir.AxisListType.X, op=mn)
                    col = oi * n_xb + (xb - GRP + 1)
                    nc.scalar.copy(strip_t[:, col:col + GRP], r8_t[:])

            for _rep in range(repeat):
                # software pipeline: emit produce(xb) before reduce(xb-1) so
                # ScalarE copies of chunk xb overlap the VectorE tree of xb-1.
                units = [(oi, o, xb) for oi, o in enumerate(("a", "b"))
                         for xb in range(n_xb)]
                wide = {}
                pend = None  # (oi, xb, cps, wide_t)
                for oi, o, xb in units:
                    if xb % GRP == 0:
                        wide[oi] = wide_pool.tile([128, GRP, BW], f16,
                                                  name="wd", tag="wd")
                    cps = produce(o, xb)
                    if pend is not None:
                        reduce_chunk(pend[0], pend[1], pend[2], pend[3])
                    pend = (oi, xb, cps, wide[oi])
                reduce_chunk(pend[0], pend[1], pend[2], pend[3])

            nc.sync.dma_start(out[:], strip_t[:])
    return nc


def _split16(v):
    """fp64 array -> (hi, lo) fp16 arrays with hi + lo ~= v."""
    hi = v.astype(np.float16)
    lo = (v - hi.astype(np.float64)).astype(np.float16)
    return hi, lo


def _features(pts, stationary):
    """pts [n,3] float64 -> [13, n] fp16 feature rows.

    stationary: [x1(3), x1(3), x2(3), s1, s2, 1, 1]
    moving:     [-2y1(3), -2y2(3), -2y1(3), 1, 1, t1, t2]
    """
    n = pts.shape[0]
    p1, p2 = _split16(pts)            # [n,3] each
    sq = np.sum(pts * pts, axis=-1)   # [n]
    s1, s2 = _split16(sq)
    one = np.ones(n, np.float16)
    if stationary:
        rows = [p1[:, 0], p1[:, 1], p1[:, 2],
                p1[:, 0], p1[:, 1], p1[:, 2],
                p2[:, 0], p2[:, 1], p2[:, 2],
                s1, s2, one, one]
    else:
        m2p1 = -2.0 * p1  # exact in fp16 (power-of-two scale)
        m2p2 = -2.0 * p2
        rows = [m2p1[:, 0], m2p1[:, 1], m2p1[:, 2],
                m2p2[:, 0], m2p2[:, 1], m2p2[:, 2],
                m2p1[:, 0], m2p1[:, 1], m2p1[:, 2],
                one, one, s1, s2]
    return np.ascontiguousarray(np.stack(rows), np.float16)


def make_in_map(xb, yb):
    """Per-core input map from one batch xb [n,3], yb [m,3]."""
    xb = np.asarray(xb, np.float64)
    yb = np.asarray(yb, np.float64)
    return {
        "l_a": _features(xb, True),
        "r_a": _features(yb, False),
        "l_b": _features(yb, True),
        "r_b": _features(xb, False),
    }


_NC_CACHE = {}


def _get_nc(n, m):
    key = (n, m)
    if key not in _NC_CACHE:
        _NC_CACHE[key] = build_nc(n, m)
    return _NC_CACHE[key]


def run_device(x, y, trace=False):
    """x [B,n,3], y [B,m,3] -> BassKernelResults with per-core strips."""
    n, m = x.shape[1], y.shape[1]
    assert x.shape[0] == N_CORES and y.shape[0] == N_CORES
    nc = _get_nc(n, m)
    in_maps = [make_in_map(x[b], y[b]) for b in range(x.shape[0])]
    return run_bass_kernel_spmd(nc, in_maps, list(range(N_CORES)), trace=trace)


def kernel(x, y):
    x = np.asarray(x)
    y = np.asarray(y)
    n, m = x.shape[1], y.shape[1]
    n_xb = n // 128
    res = run_device(x, y)
    s2_tot = 0.0  # sum over per-x mins  (reference dist2, min over j)
    s1_tot = 0.0  # sum over per-y mins  (reference dist1, min over i)
    for b in range(x.shape[0]):
        strip = res.results[b]["strip"].astype(np.float64)
        s2_tot += strip[:, :n_xb].sum()
        s1_tot += strip[:, n_xb:].sum()
    out = s1_tot / (x.shape[0] * m) + s2_tot / (x.shape[0] * n)
    return np.float32(out)
